# revision 1
# baseline (speedup 1.0000x reference)
"""Causal self-attention kernel for Trainium2, 8 NeuronCores, data-parallel over batch.

v3: bf16 dataflow (4x PE throughput vs fp32), DMA-transpose input, merged
normalization.

Problem: B=4096 independent attentions, T=64, DIM=128, 4 heads of 32;
y = proj(attn(x)). k_in / v_in unused (overwritten internally) -> never shipped.

Per core: 512 batches = 32768 tokens, 64 mega-tiles of 512 tokens (8 batches).
Per mega:
  - x^T [128, 512] bf16 delivered straight from DRAM by the HWDGE xbar
    transpose (host pre-casts x to bf16); no PE transpose, no eviction.
  - q^T/k^T head-pair tiles + v natural, all bf16 matmuls at 1 cycle/row
    (4x the fp32 rate); q-bias folded into the DVE evictions.
  - scores^T [keys, (pair,head,query)] seeded with the causal mask (-80) by
    an identity matmul; bf16 score matmuls accumulate on top; ACT exp per
    512-col bank half so the softmax chain pipelines.
  - normalization: rb = blockhalf^T @ attn_u broadcasts denominators to all
    128 key partitions directly (no separate sums stage); reciprocal on DVE
    (bf16 out); the normalize multiply runs on the otherwise-idle GPSIMD.
  - y^T per-batch-half tiles via ap=64 matmuls; projection = two K=64
    accumulating matmuls per 128-token chunk; bias + eviction in one DVE add.
  Bias algebra: k-bias dropped (softmax-invariant); v-bias folded into proj
  bias on host.
Schedule: 3-stage software pipeline (qkv for mega m+1, scores/softmax for m,
yT/proj/output for m-1) so every PE-stream operand is >= 1 iteration old;
x^T prefetched 8 megas ahead; output DMA on the SP HWDGE queue.
PSUM: 8 banks static via tag-sharing (qt, kt, v+yt0, yt1, sc x2, rb+yf x2).
"""

import sys

for _p in ("/opt/trn_rl_repo", "/root/.axon_site/_ro/trn_rl_repo"):
    if _p not in sys.path:
        sys.path.insert(0, _p)

from contextlib import ExitStack

import ml_dtypes
import numpy as np

import concourse.bass as bass
import concourse.tile as tile
from concourse import bacc
from concourse import mybir
from concourse.bass_utils import run_bass_kernel_spmd

F32 = mybir.dt.float32
BF16 = mybir.dt.bfloat16

B, T, D, H, HS = 4096, 64, 128, 4, 32
NCORES = 8
BC = B // NCORES            # 512 batches per core
TOK = BC * T                # 32768 tokens per core
MEGA = 512                  # tokens per mega-tile (8 batches = 4 batch-pairs)
SCALE = 1.0 / float(np.sqrt(HS))
NEG = -80.0                 # additive causal mask (exp(-80) ~ 0)

_CACHE = {}
LAST_RESULT = None


def _bf16(a):
    return np.asarray(a, dtype=np.float32).astype(ml_dtypes.bfloat16)


def _host_consts(W_attn, b_attn, W_proj, b_proj):
    """Constant tiles: one bf16 pack [128, Nb] and one fp32 pack [128, Nf]."""
    Wq = np.ascontiguousarray(W_attn[:, 0:128] * SCALE)          # [128,128]
    Wk = np.ascontiguousarray(W_attn[:, 128:256])
    Wv = np.ascontiguousarray(W_attn[:, 256:384])
    bqs = b_attn[0:128] * SCALE
    bv = b_attn[256:384]
    bp_eff = b_proj + bv @ W_proj                                # [128]

    ident = np.eye(128, dtype=np.float32)
    # causal mask, transposed-scores layout: [row=b*64+kk, col=...*64+qq]
    kk = np.arange(64).reshape(64, 1)
    qq = np.arange(64).reshape(1, 64)
    m0 = np.where(kk <= qq, 0.0, NEG).astype(np.float32)         # [64,64]
    maskT = np.tile(np.tile(m0, (2, 1)), (1, 16))                # [128, 1024]
    # blockhalf: B[i,j] = 1 if i//64 == j//64 (same batch within the pair)
    bhalf = np.zeros((128, 128), dtype=np.float32)
    bhalf[0:64, 0:64] = 1.0
    bhalf[64:128, 64:128] = 1.0

    # wps: [64, 256] K=64-split proj weight (rows 64..127 zero-padded)
    wps = np.zeros((128, 256), dtype=np.float32)
    wps[0:64, 0:128] = W_proj[0:64, :]
    wps[0:64, 128:256] = W_proj[64:128, :]
    cb_parts = [
        ("wq", Wq),
        ("wk", Wk),
        ("wv", Wv),
        ("wps", wps),
        ("bhalf", bhalf),
        ("ident", ident),
        ("maskT", maskT),
    ]
    cb = np.concatenate([np.asarray(a, dtype=np.float32) for _, a in cb_parts],
                        axis=1).astype(ml_dtypes.bfloat16)
    cb_off = {}
    off = 0
    for name, a in cb_parts:
        cb_off[name] = (off, a.shape[1])
        off += a.shape[1]

    # bq2[:, j]: q-bias for head-pair j on partitions 0..63
    bq2 = np.zeros((128, 2), dtype=np.float32)
    bq2[:, 0] = bqs
    biasP = np.ascontiguousarray(
        np.broadcast_to(bp_eff.reshape(1, 1, 128), (128, 4, 128)).reshape(128, 512)
    ).astype(np.float32)
    cf_parts = [("bq2", bq2), ("biasP", biasP)]
    cf = np.concatenate([a for _, a in cf_parts], axis=1).astype(np.float32)
    cf_off = {}
    off = 0
    for name, a in cf_parts:
        cf_off[name] = (off, a.shape[1])
        off += a.shape[1]
    return cb, cb_off, cf, cf_off


def _build_program(cb_off, cb_cols, cf_off, cf_cols, ntok=TOK):
    nmega = ntok // MEGA
    nc = bacc.Bacc()
    x_p = nc.declare_dram_parameter("x", [(ntok // MEGA) * 128, MEGA], BF16, isOutput=False)
    y_p = nc.declare_dram_parameter("y", [ntok, D], F32, isOutput=True)
    cb_p = nc.declare_dram_parameter("cb", [128, cb_cols], BF16, isOutput=False)
    cf_p = nc.declare_dram_parameter("cf", [128, cf_cols], F32, isOutput=False)

    xT_rows = x_p.rearrange("(m d) t -> m d t", d=128)
    # token index: t = m*512 + c*128 + p  (c = chunk = batch-pair, p = 64b+q)
    y_v = y_p.rearrange("(m c p) d -> m p c d", c=4, p=128)

    # scores^T column map: col = 512*(h%2) + 128*pp + 64*(h//2) + q
    # (bank = h%2; kt/qt stationary row base r0 = 32*(h%2) is then unique per
    # bank -- concurrent different-row-strip writes to a bank fault)
    def scol(pp, h):
        return 512 * (h % 2) + 128 * pp + 64 * (h // 2)

    Copy = mybir.ActivationFunctionType.Copy
    Exp = mybir.ActivationFunctionType.Exp

    with nc.allow_low_precision(reason="bf16 attention dataflow"), \
            tile.TileContext(nc) as tc, ExitStack() as ctx:
        cpool = ctx.enter_context(tc.tile_pool(name="consts", bufs=1))
        sb = ctx.enter_context(tc.tile_pool(name="sb", bufs=2))
        ps = ctx.enter_context(tc.tile_pool(name="ps", bufs=1, space="PSUM"))

        cball = cpool.tile([128, cb_cols], BF16, tag="cb_all")
        nc.sync.dma_start(out=cball[:], in_=cb_p[:])
        cfall = cpool.tile([128, cf_cols], F32, tag="cf_all")
        nc.sync.dma_start(out=cfall[:], in_=cf_p[:])
        CB = {n: cball[:, o: o + w] for n, (o, w) in cb_off.items()}
        CF = {n: cfall[:, o: o + w] for n, (o, w) in cf_off.items()}

        PREFETCH = 8
        xT_tiles = {}

        def fetch_x(mm):
            if mm >= nmega:
                return
            t = sb.tile([128, MEGA], BF16, tag="xT", bufs=PREFETCH + 1, name=f"xT{mm}")
            nc.sync.dma_start(out=t[:], in_=xT_rows[mm])
            xT_tiles[mm] = t

        for mm in range(PREFETCH):
            fetch_x(mm)

        def qkv_stage(m):
            """q^T/k^T head-pair tiles + v natural (all bf16 evictions)."""
            xT = xT_tiles.pop(m)
            qt_ps = ps.tile([128, 512], F32, tag="qt")
            nc.tensor.matmul(qt_ps[:], CB["wq"], xT[:], start=True, stop=True)
            qtA = sb.tile([128, 512], BF16, tag="qtA", bufs=3)
            nc.vector.tensor_scalar_add(qtA[:], qt_ps[:], CF["bq2"][:, 0:1])
            qtB = sb.tile([64, 512], BF16, tag="qtB", bufs=3)
            nc.sync.dma_start(out=qtB[:], in_=qtA[64:128, :])
            kt_ps = ps.tile([128, 512], F32, tag="kt")
            nc.tensor.matmul(kt_ps[:], CB["wk"], xT[:], start=True, stop=True)
            ktA = sb.tile([128, 512], BF16, tag="ktA", bufs=3)
            nc.scalar.activation(ktA[:], kt_ps[:], Copy)
            ktB = sb.tile([64, 512], BF16, tag="ktB", bufs=3)
            nc.sync.dma_start(out=ktB[:], in_=ktA[64:128, :])
            v_ps = ps.tile([128, 4, 128], F32, tag="pvy")
            for c in range(4):
                nc.tensor.matmul(
                    v_ps[:, c, :], xT[:, c * 128:(c + 1) * 128], CB["wv"],
                    start=True, stop=True,
                )
            v_s = sb.tile([128, 4, 128], BF16, tag="v_s", bufs=3)
            nc.scalar.activation(v_s[:], v_ps[:], Copy)
            return {"qtA": qtA, "qtB": qtB, "ktA": ktA, "ktB": ktB, "v_s": v_s}

        def score_stage(qk):
            """Masked scores + softmax. Bank halves (h%2); exp of bank0 runs
            while the PE streams bank1's scores, so the rb matmuls (emitted
            after both banks) never wait on the ACT exp."""
            sc_ps = ps.tile([128, 1024], F32, tag="sc")
            attn_u = sb.tile([128, 1024], BF16, tag="attn_u", bufs=3)
            for half in range(2):
                cc = half * 512
                nc.tensor.matmul(
                    sc_ps[:, cc:cc + 512],
                    CB["ident"],
                    CB["maskT"][:, cc:cc + 512],
                    start=True, stop=False, skip_group_check=True,
                )
                for h in (half, half + 2):            # heads in this bank
                    qt = (qk["qtA"], qk["qtB"])[h // 2]
                    kt = (qk["ktA"], qk["ktB"])[h // 2]
                    r0 = 32 * (h % 2)
                    for pp in range(4):
                        for b in (0, 1):
                            bb = pp * 2 + b
                            c0 = scol(pp, h)
                            nc.tensor.matmul(
                                sc_ps[b * 64:(b + 1) * 64, c0:c0 + 64],
                                kt[r0:r0 + 32, bb * 64:(bb + 1) * 64],
                                qt[r0:r0 + 32, bb * 64:(bb + 1) * 64],
                                start=False,
                                stop=(h == half + 2 and pp == 3 and b == 1),
                                skip_group_check=True,
                            )
                nc.scalar.activation(attn_u[:, cc:cc + 512], sc_ps[:, cc:cc + 512], Exp)
            rb_ps = ps.tile([128, 1024], F32, tag="rbyf")
            rec = sb.tile([128, 1024], BF16, tag="rec", bufs=3)
            attn_n = sb.tile([128, 1024], BF16, tag="attn_n", bufs=3)
            for half in range(2):
                cc = half * 512
                nc.tensor.matmul(
                    rb_ps[:, cc:cc + 512], CB["bhalf"], attn_u[:, cc:cc + 512],
                    start=True, stop=True,
                )
                nc.vector.reciprocal(rec[:, cc:cc + 512], rb_ps[:, cc:cc + 512])
                nc.gpsimd.tensor_mul(
                    attn_n[:, cc:cc + 512], attn_u[:, cc:cc + 512], rec[:, cc:cc + 512]
                )
            return attn_n

        def out_stage(m, qk, attn_n):
            """y^T tiles, reassembly, projection, bias, DMA out."""
            v_s = qk["v_s"]
            ytb = [
                ps.tile([64, 2, 4, 64], F32, tag="pvy", name="yt0"),
                ps.tile([64, 2, 4, 64], F32, tag="yt1", name="yt1"),
            ]
            for half in range(2):
                for b in (0, 1):
                    for X in (0, 1):
                        h = 2 * X + half
                        for pp in range(4):
                            c0 = scol(pp, h)
                            nc.tensor.matmul(
                                ytb[b][32 * half:32 * (half + 1), X, pp, :],
                                v_s[b * 64:(b + 1) * 64, pp, 32 * h:32 * (h + 1)],
                                attn_n[b * 64:(b + 1) * 64, c0:c0 + 64],
                                start=True, stop=True,
                            )
            yTs = []
            for X in (0, 1):
                yTX = sb.tile([64, 4, 2, 64], BF16, tag=f"yT{X}", bufs=3)
                yTs.append(yTX)
                nc.scalar.activation(yTX[:, :, 0, :], ytb[0][:, X], Copy)
                nc.vector.tensor_copy(yTX[:, :, 1, :], ytb[1][:, X])

            yf_ps = ps.tile([128, 4, 128], F32, tag="rbyf")
            for c in range(4):
                for X in (0, 1):
                    nc.tensor.matmul(
                        yf_ps[:, c, :],
                        yTs[X][:].rearrange("f pp b q -> f (pp b q)")[:, c * 128:(c + 1) * 128],
                        CB["wps"][0:64, 128 * X:128 * (X + 1)],
                        start=(X == 0), stop=(X == 1),
                    )
            y_out = sb.tile([128, 4, 128], F32, tag="y_out", bufs=3)
            nc.vector.tensor_add(y_out[:], yf_ps[:], CF["biasP"].rearrange("p (c d) -> p c d", c=4))
            # out-DMA rides the Activation HWDGE queue: its dependency resolves
            # late, and on the SP queue it would head-of-line block the next
            # iteration's prefetch and shift DMAs
            # out-DMA on the ACT HWDGE queue: SP carries 3 load/shift issues per
            # iteration and was pacing with 4
            nc.scalar.dma_start(out=y_v[m], in_=y_out[:])

        # 3-stage software pipeline: iteration i runs scores/softmax for mega
        # i-1, qkv for mega i, and the yT/proj/output for mega i-2 -- every
        # value consumed on the PE stream is at least one full iteration old.
        qk_st = {}
        an_st = {}
        for i in range(nmega + 2):
            if i < nmega:
                fetch_x(i + PREFETCH)
                qk_st[i] = qkv_stage(i)
            if 1 <= i <= nmega:
                an_st[i - 1] = score_stage(qk_st[i - 1])
            if i >= 2:
                m = i - 2
                out_stage(m, qk_st.pop(m), an_st.pop(m))
    nc.compile()
    return nc


def _cast_bf16_fast(x):
    """fp32 -> bf16 round-to-nearest-even via bit ops (faster than astype)."""
    u = x.view(np.uint32)
    r = ((u >> 16) & 1) + np.uint32(0x7FFF)
    return ((u + r) >> 16).astype(np.uint16).view(ml_dtypes.bfloat16)


def kernel(x, k_in, v_in, W_attn, b_attn, W_proj, b_proj):
    x = np.asarray(x, dtype=np.float32)
    cb, cb_off, cf, cf_off = _host_consts(
        np.asarray(W_attn, dtype=np.float32),
        np.asarray(b_attn, dtype=np.float32),
        np.asarray(W_proj, dtype=np.float32),
        np.asarray(b_proj, dtype=np.float32),
    )
    key = "prog"
    if key not in _CACHE:
        _CACHE[key] = _build_program(cb_off, cb.shape[1], cf_off, cf.shape[1])
    nc = _CACHE[key]

    xb = _cast_bf16_fast(np.ascontiguousarray(x.reshape(B * T, D)))
    # pre-transpose per mega on host: [TOK, 128] -> [nmega, 128, 512]
    xbt = np.ascontiguousarray(
        xb.reshape(NCORES, TOK // MEGA, MEGA, D).transpose(0, 1, 3, 2)
    )
    in_maps = []
    for i in range(NCORES):
        shard = xbt[i].reshape((TOK // MEGA) * 128, MEGA)
        in_maps.append({"x": shard, "cb": cb, "cf": cf})

    res = run_bass_kernel_spmd(nc, in_maps, list(range(NCORES)))
    global LAST_RESULT
    LAST_RESULT = res
    outs = [res.results[i]["y"].reshape(BC, T, D) for i in range(NCORES)]
    return np.concatenate(outs, axis=0)


if __name__ == "__main__":
    rng = np.random.default_rng(0)
    Bs = 64  # small smoke test: one core, 8 megas
    ntok = Bs * T
    xs = rng.standard_normal((Bs, T, D), dtype=np.float32)
    Wa = rng.standard_normal((D, 3 * D), dtype=np.float32) / np.sqrt(D)
    ba = rng.standard_normal(3 * D, dtype=np.float32) * 0.01
    Wp = rng.standard_normal((D, D), dtype=np.float32) / np.sqrt(D)
    bp = rng.standard_normal(D, dtype=np.float32) * 0.01

    cb, cb_off, cf, cf_off = _host_consts(Wa, ba, Wp, bp)
    nc = _build_program(cb_off, cb.shape[1], cf_off, cf.shape[1], ntok=ntok)
    xb = _cast_bf16_fast(np.ascontiguousarray(xs.reshape(ntok, D)))
    xbt = np.ascontiguousarray(
        xb.reshape(ntok // MEGA, MEGA, D).transpose(0, 2, 1)
    ).reshape((ntok // MEGA) * 128, MEGA)
    res = run_bass_kernel_spmd(nc, [{"x": xbt, "cb": cb, "cf": cf}], [0])
    y = res.results[0]["y"].reshape(Bs, T, D)

    # numpy reference
    def ref(x):
        qkv = x @ Wa + ba
        q, k, v = np.split(qkv, 3, axis=2)

        def heads(u):
            return u.reshape(Bs, T, H, HS).transpose(0, 2, 1, 3)

        q, k, v = heads(q), heads(k), heads(v)
        s = np.einsum('bhqd,bhkd->bhqk', q, k) / np.sqrt(HS)
        mask = np.tril(np.ones((T, T), dtype=bool))
        s = np.where(mask, s, -1e30)
        e = np.exp(s - s.max(axis=-1, keepdims=True))
        a = e / e.sum(axis=-1, keepdims=True)
        o = np.einsum('bhqk,bhkd->bhqd', a, v)
        o = o.transpose(0, 2, 1, 3).reshape(Bs, T, D)
        return o @ Wp + bp

    want = ref(xs)
    err = np.linalg.norm(y - want) / np.linalg.norm(want)
    print("smoke rel err:", err)



# revision 7
# speedup vs baseline: 1.1670x; 1.1670x over previous
"""Causal self-attention kernel for Trainium2, 8 NeuronCores, data-parallel over batch.

v7: 2-head-packed score matmuls, GPSIMD mask, yT-layout denominators with
fused divide-normalize, single-matmul projection, bf16 transposed output.

Problem: B=4096 independent attentions, T=64, DIM=128, 4 heads of 32;
y = proj(attn(x)). k_in / v_in unused (overwritten internally) -> never shipped.

Per core: 512 batches = 32768 tokens, 64 megas of 512 tokens (8 batches).
Per mega (PE cycles, bf16 1 cyc/col):
  - qkv: qt/kt [dim, tok] via const-W stationary (512+512); v natural [tok, dim]
    via xT-chunk stationary (512).
  - scores: TWO heads per matmul: stationary ktE block-diag [64 K, 128 M]
    (M = 2 heads x 64 keys), moving qtA natural 64-row windows -> out
    [128 = 2h x keys, 64 q]; 16 matmuls x 64 cols = 1024.
  - mask: multiplicative 0/1 bf16 pattern on GPSIMD after one full-width exp.
  - rbY: denominators broadcast directly in yT row layout (ones stationary,
    attn moving; 2 x 512). yt: 32 matmuls x 64 = 2048. proj: ONE 512-col
    matmul (const Wp stationary) -> yfT [dout, tok].
  - normalize fused into yt eviction: ytS = yt_ps / rbY_ps (DVE divide).
  - output: ACT Identity+bias eviction to bf16, DMA [dout, tok]; host
    transposes and upcasts.
Evictions balanced across DVE/ACT/Pool; ktE built with 4x-mode bf16 copies.
Schedule: 3-stage software pipeline as v3; x^T prefetched 8 megas ahead.
PSUM: 8 banks static (qt, kt, v, sc x2, rbY, yt, yfT).
"""

import sys

for _p in ("/opt/trn_rl_repo", "/root/.axon_site/_ro/trn_rl_repo"):
    if _p not in sys.path:
        sys.path.insert(0, _p)

from contextlib import ExitStack

import ml_dtypes
import numpy as np

import concourse.bass as bass
import concourse.tile as tile
from concourse import bacc
from concourse import mybir
from concourse.bass_utils import run_bass_kernel_spmd

F32 = mybir.dt.float32
BF16 = mybir.dt.bfloat16

B, T, D, H, HS = 4096, 64, 128, 4, 32
NCORES = 8
BC = B // NCORES            # 512 batches per core
TOK = BC * T                # 32768 tokens per core
MEGA = 512                  # tokens per mega-tile (8 batches)
SCALE = 1.0 / float(np.sqrt(HS))

_CACHE = {}
LAST_RESULT = None


def _bf16(a):
    return np.asarray(a, dtype=np.float32).astype(ml_dtypes.bfloat16)


def _host_consts(W_attn, b_attn, W_proj, b_proj):
    """bf16 pack cb [128, *] and fp32 pack cf [128, *]."""
    Wq = np.ascontiguousarray(W_attn[:, 0:128] * SCALE)          # [128,128]
    Wk = np.ascontiguousarray(W_attn[:, 128:256])                # k-bias dropped
    Wv = np.ascontiguousarray(W_attn[:, 256:384])
    Wp = np.ascontiguousarray(W_proj)                            # [d, dout]
    bqs = b_attn[0:128] * SCALE
    bv = b_attn[256:384]
    bp_eff = b_proj + bv @ W_proj                                # [128] (dout)

    # maskbit [128, 1024]: row = s*64+k, col = (X, bb, q) -> [k <= q]
    kk = np.arange(64).reshape(64, 1)
    qq = np.arange(64).reshape(1, 64)
    m0 = np.where(kk <= qq, 1.0, 0.0).astype(np.float32)         # [64, 64]
    maskbit = np.tile(m0, (2, 16))                                # [128, 1024]

    # onesY [128, 64]: [s'*64+k, 32*s+i] = (s' == s)
    onesY = np.zeros((128, 64), dtype=np.float32)
    onesY[0:64, 0:32] = 1.0
    onesY[64:128, 32:64] = 1.0

    cb_parts = [("wq", Wq), ("wk", Wk), ("wv", Wv), ("wp", Wp),
                ("onesY", onesY), ("maskbit", maskbit)]
    cb = np.concatenate([np.asarray(a, dtype=np.float32) for _, a in cb_parts],
                        axis=1).astype(ml_dtypes.bfloat16)
    cb_off = {}
    off = 0
    for name, a in cb_parts:
        cb_off[name] = (off, a.shape[1])
        off += a.shape[1]

    # fp32: per-partition scalars: bq (qdim rows), bpE (dout rows)
    cf_parts = [("bq", bqs.reshape(128, 1)), ("bpE", bp_eff.reshape(128, 1))]
    cf = np.concatenate([a for _, a in cf_parts], axis=1).astype(np.float32)
    cf_off = {}
    off = 0
    for name, a in cf_parts:
        cf_off[name] = (off, a.shape[1])
        off += a.shape[1]
    return cb, cb_off, cf, cf_off


def _build_program(cb_off, cb_cols, cf_off, cf_cols, ntok=TOK):
    nmega = ntok // MEGA
    nc = bacc.Bacc()
    x_p = nc.declare_dram_parameter("x", [nmega * 128, MEGA], BF16, isOutput=False)
    # output transposed per mega: [dout, tok]; host untransposes
    y_p = nc.declare_dram_parameter("y", [nmega * 128, MEGA], BF16, isOutput=True)
    cb_p = nc.declare_dram_parameter("cb", [128, cb_cols], BF16, isOutput=False)
    cf_p = nc.declare_dram_parameter("cf", [128, cf_cols], F32, isOutput=False)

    xT_rows = x_p.rearrange("(m d) t -> m d t", d=128)
    yT_rows = y_p.rearrange("(m d) t -> m d t", d=128)

    Copy = mybir.ActivationFunctionType.Copy
    Identity = mybir.ActivationFunctionType.Identity
    Exp = mybir.ActivationFunctionType.Exp
    MULT = mybir.AluOpType.mult

    with nc.allow_low_precision(reason="bf16 attention dataflow"), \
            tile.TileContext(nc) as tc, ExitStack() as ctx:
        cpool = ctx.enter_context(tc.tile_pool(name="consts", bufs=1))
        sb = ctx.enter_context(tc.tile_pool(name="sb", bufs=2))
        ps = ctx.enter_context(tc.tile_pool(name="ps", bufs=1, space="PSUM"))

        cball = cpool.tile([128, cb_cols], BF16, tag="cb_all")
        nc.sync.dma_start(out=cball[:], in_=cb_p[:])
        cfall = cpool.tile([128, cf_cols], F32, tag="cf_all")
        nc.sync.dma_start(out=cfall[:], in_=cf_p[:])
        CB = {n: cball[:, o: o + w] for n, (o, w) in cb_off.items()}
        CF = {n: cfall[:, o: o + w] for n, (o, w) in cf_off.items()}

        PREFETCH = 8
        KTE_BUFS = 3
        xT_tiles = {}

        def fetch_x(mm):
            if mm >= nmega:
                return
            t = sb.tile([128, MEGA], BF16, tag="xT", bufs=PREFETCH + 1, name=f"xT{mm}")
            nc.sync.dma_start(out=t[:], in_=xT_rows[mm])
            xT_tiles[mm] = t

        for mm in range(PREFETCH):
            fetch_x(mm)

        # pre-zero ktE rotating buffers (off-diagonal blocks stay zero forever)
        for zz in range(KTE_BUFS):
            zt = sb.tile([128, 8, 128], BF16, tag="ktE", bufs=KTE_BUFS, name=f"ktEz{zz}")
            nc.vector.memset(zt[:], 0.0)

        def qkv_stage(m):
            xT = xT_tiles.pop(m)
            # q^T [qdim, tok]
            qt_ps = ps.tile([128, 512], F32, tag="qt")
            nc.tensor.matmul(qt_ps[:], CB["wq"], xT[:], start=True, stop=True)
            qtA = sb.tile([128, 512], BF16, tag="qtA", bufs=3)
            nc.vector.tensor_scalar_add(qtA[:], qt_ps[:], CF["bq"])
            # k^T [kdim, tok]
            kt_ps = ps.tile([128, 512], F32, tag="kt")
            nc.tensor.matmul(kt_ps[:], CB["wk"], xT[:], start=True, stop=True)
            ktN = sb.tile([128, 8, 64], BF16, tag="ktN", bufs=3)
            nc.scalar.activation(ktN[:], kt_ps[:].rearrange("p (b k) -> p b k", b=8), Copy)
            # ktE block-diag [128, 8, 128]: rows 32h..32h+32 hold head h's
            # [hs, keys] block at col half h%2; both X share the tile
            # (X0 = rows 0:64, X1 = rows 64:128). 4x-mode bf16 copies.
            ktE = sb.tile([128, 8, 128], BF16, tag="ktE", bufs=KTE_BUFS)
            for h in range(4):
                r0 = 32 * h
                c0 = 64 * (h % 2)
                nc.vector.tensor_copy(ktE[r0:r0 + 32, :, c0:c0 + 64], ktN[r0:r0 + 32, :, :])
            # v natural [tok, d] per 128-token chunk
            v_ps = ps.tile([128, 4, 128], F32, tag="pv")
            for c in range(4):
                nc.tensor.matmul(
                    v_ps[:, c, :], xT[:, c * 128:(c + 1) * 128], CB["wv"],
                    start=True, stop=True,
                )
            v_s = sb.tile([128, 4, 128], BF16, tag="v_s", bufs=3)
            nc.scalar.activation(v_s[:], v_ps[:], Copy)
            # v_sw = v_s with partition halves swapped (for yt matmuls where
            # the attn row-half parity differs from the batch parity)
            v_sw = sb.tile([128, 4, 128], BF16, tag="v_sw", bufs=3)
            nc.sync.dma_start(out=v_sw[0:64, :, :], in_=v_s[64:128, :, :])
            nc.sync.dma_start(out=v_sw[64:128, :, :], in_=v_s[0:64, :, :])
            return {"qtA": qtA, "ktE": ktE, "v_s": v_s, "v_sw": v_sw}

        def score_stage(qk):
            """Packed scores + exp + mask + yT-layout denominators."""
            qtA, ktE = qk["qtA"], qk["ktE"]
            sc_ps = ps.tile([128, 2, 8, 64], F32, tag="sc")
            for X in range(2):
                r0 = 64 * X
                for bb in range(8):
                    nc.tensor.matmul(
                        sc_ps[:, X, bb, :],
                        ktE[r0:r0 + 64, bb, :],
                        qtA[r0:r0 + 64, bb * 64:(bb + 1) * 64],
                        start=True, stop=True,
                    )
            attn_r = sb.tile([128, 1024], BF16, tag="attn_r", bufs=3)
            nc.scalar.activation(attn_r[:], sc_ps[:].rearrange("p x b q -> p (x b q)"), Exp)
            attn_u = sb.tile([128, 1024], BF16, tag="attn_u", bufs=3)
            nc.gpsimd.tensor_tensor(attn_u[:], attn_r[:], CB["maskbit"], op=MULT)
            # denominators straight into yT row layout: row 64X+32s+i = head 2X+s
            rb_ps = ps.tile([128, 512], F32, tag="rbY")
            for X in range(2):
                nc.tensor.matmul(
                    rb_ps[64 * X:64 * X + 64, :], CB["onesY"],
                    attn_u[:, 512 * X:512 * X + 512],
                    start=True, stop=True,
                )
            rec = sb.tile([128, 512], BF16, tag="rec", bufs=3)
            nc.vector.reciprocal(rec[:], rb_ps[:])
            return attn_u, rec

        def out_stage(m, qk, attn_u, rec):
            yt_ps = ps.tile([128, 512], F32, tag="yt")
            for h in range(4):
                X, s = h // 2, h % 2
                for bb in range(8):
                    # operand bases must match: batch bb's keys sit at rows
                    # 64*(bb%2) of v_s; attn rows for head h sit at 64*s.
                    # s == bb%2 -> v_s, else the half-swapped copy v_sw.
                    vt = qk["v_s"] if s == (bb % 2) else qk["v_sw"]
                    nc.tensor.matmul(
                        yt_ps[32 * h:32 * h + 32, bb * 64:(bb + 1) * 64],
                        vt[64 * s:64 * s + 64, bb // 2, 32 * h:32 * h + 32],
                        attn_u[64 * s:64 * s + 64, 512 * X + 64 * bb: 512 * X + 64 * bb + 64],
                        start=True, stop=True,
                        tile_position=(64 * s, 32 * h),
                    )
            # normalize fused into the yt eviction: ytS = yt_ps * rec
            ytS = sb.tile([128, 512], BF16, tag="ytS", bufs=3)
            nc.vector.tensor_tensor(ytS[:], yt_ps[:], rec[:], op=MULT)
            # proj: yfT [dout, tok] = Wp^T @ ytS, one 512-col matmul
            yf_ps = ps.tile([128, 512], F32, tag="yfT")
            nc.tensor.matmul(yf_ps[:], CB["wp"], ytS[:], start=True, stop=True)
            y_out = sb.tile([128, 512], BF16, tag="y_out", bufs=3)
            nc.scalar.activation(y_out[:], yf_ps[:], Identity, bias=CF["bpE"])
            nc.scalar.dma_start(out=yT_rows[m], in_=y_out[:])

        # 3-stage software pipeline (see v3 docstring)
        qk_st = {}
        an_st = {}
        for i in range(nmega + 2):
            if i < nmega:
                fetch_x(i + PREFETCH)
                qk_st[i] = qkv_stage(i)
            if 1 <= i <= nmega:
                an_st[i - 1] = score_stage(qk_st[i - 1])
            if i >= 2:
                m = i - 2
                out_stage(m, qk_st.pop(m), *an_st.pop(m))
    nc.compile()
    return nc


def _cast_bf16_fast(x):
    """fp32 -> bf16 round-to-nearest-even via bit ops (faster than astype)."""
    u = x.view(np.uint32)
    r = ((u >> 16) & 1) + np.uint32(0x7FFF)
    return ((u + r) >> 16).astype(np.uint16).view(ml_dtypes.bfloat16)


def kernel(x, k_in, v_in, W_attn, b_attn, W_proj, b_proj):
    x = np.asarray(x, dtype=np.float32)
    cb, cb_off, cf, cf_off = _host_consts(
        np.asarray(W_attn, dtype=np.float32),
        np.asarray(b_attn, dtype=np.float32),
        np.asarray(W_proj, dtype=np.float32),
        np.asarray(b_proj, dtype=np.float32),
    )
    key = "prog"
    if key not in _CACHE:
        _CACHE[key] = _build_program(cb_off, cb.shape[1], cf_off, cf.shape[1])
    nc = _CACHE[key]

    xb = _cast_bf16_fast(np.ascontiguousarray(x.reshape(B * T, D)))
    # pre-transpose per mega on host: [TOK, 128] -> [nmega, 128, 512]
    xbt = np.ascontiguousarray(
        xb.reshape(NCORES, TOK // MEGA, MEGA, D).transpose(0, 1, 3, 2)
    )
    in_maps = []
    for i in range(NCORES):
        shard = xbt[i].reshape((TOK // MEGA) * 128, MEGA)
        in_maps.append({"x": shard, "cb": cb, "cf": cf})

    res = run_bass_kernel_spmd(nc, in_maps, list(range(NCORES)))
    global LAST_RESULT
    LAST_RESULT = res
    outs = []
    for i in range(NCORES):
        yT = np.asarray(res.results[i]["y"]).view(ml_dtypes.bfloat16)
        yT = yT.reshape(TOK // MEGA, 128, MEGA).transpose(0, 2, 1)  # [m, tok, d]
        outs.append(yT.astype(np.float32).reshape(BC, T, D))
    return np.concatenate(outs, axis=0)


if __name__ == "__main__":
    rng = np.random.default_rng(0)
    Bs = 64  # small smoke test: one core, 8 megas
    ntok = Bs * T
    xs = rng.standard_normal((Bs, T, D), dtype=np.float32)
    Wa = rng.standard_normal((D, 3 * D), dtype=np.float32) / np.sqrt(D)
    ba = rng.standard_normal(3 * D, dtype=np.float32) * 0.01
    Wp = rng.standard_normal((D, D), dtype=np.float32) / np.sqrt(D)
    bp = rng.standard_normal(D, dtype=np.float32) * 0.01

    cb, cb_off, cf, cf_off = _host_consts(Wa, ba, Wp, bp)
    nc = _build_program(cb_off, cb.shape[1], cf_off, cf.shape[1], ntok=ntok)
    xb = _cast_bf16_fast(np.ascontiguousarray(xs.reshape(ntok, D)))
    xbt = np.ascontiguousarray(
        xb.reshape(ntok // MEGA, MEGA, D).transpose(0, 2, 1)
    ).reshape((ntok // MEGA) * 128, MEGA)
    res = run_bass_kernel_spmd(nc, [{"x": xbt, "cb": cb, "cf": cf}], [0])
    yT = np.asarray(res.results[0]["y"]).view(ml_dtypes.bfloat16)
    y = yT.reshape(ntok // MEGA, 128, MEGA).transpose(0, 2, 1).astype(np.float32)
    y = y.reshape(Bs, T, D)

    # numpy reference
    def ref(x):
        qkv = x @ Wa + ba
        q, k, v = np.split(qkv, 3, axis=2)

        def heads(u):
            return u.reshape(Bs, T, H, HS).transpose(0, 2, 1, 3)

        q, k, v = heads(q), heads(k), heads(v)
        s = np.einsum('bhqd,bhkd->bhqk', q, k) / np.sqrt(HS)
        mask = np.tril(np.ones((T, T), dtype=bool))
        s = np.where(mask, s, -1e30)
        e = np.exp(s - s.max(axis=-1, keepdims=True))
        a = e / e.sum(axis=-1, keepdims=True)
        o = np.einsum('bhqk,bhkd->bhqd', a, v)
        o = o.transpose(0, 2, 1, 3).reshape(Bs, T, D)
        return o @ Wp + bp

    want = ref(xs)
    err = np.linalg.norm(y - want) / np.linalg.norm(want)
    print("smoke rel err:", err)
